# revision 4
# baseline (speedup 1.0000x reference)
"""MoE layer (top-2 of 8 experts, T=4096, D=1024, F=4096) on 8 Trainium2
NeuronCores, expert-parallel.

Sharding: x is replicated to all cores; core c holds expert c's weights.
Each core computes the router in fp32 (token routing must match the fp32
reference bit-for-bit at the top-k decision), runs its expert's FFN in bf16
over all tokens, scales each token row by its combine weight (zero when the
token is not routed to this expert), and an fp32 ReduceScatter sums the
per-expert partials so core c ends up with token rows [512c, 512c+512).
The host concatenates the 8 shards. The load-balance aux loss is computed
on-device (identically on every core) from per-expert prob sums and top-2
counts.

Self-contained: only needs numpy / ml_dtypes / concourse (the Bass stack
present in the execution container).
"""
import numpy as np
import ml_dtypes

import concourse.bass as bass
import concourse.mybir as mybir
import concourse.tile as tile

F32 = mybir.dt.float32
BF16 = mybir.dt.bfloat16
AF = mybir.ActivationFunctionType
ALU = mybir.AluOpType

# problem dims (hardcoded per task spec)
T, D, F, E = 4096, 1024, 4096, 8
N_CORES = 8
KT = D // 128          # contraction tiles for x @ W1 and the router
FT = F // 128          # contraction tiles for h @ W2
TCH = 512              # tokens per FFN chunk
NCH = T // TCH
TSHARD = T // N_CORES


def _split_excess_waits(nc, max_keep=1):
    """This container's walrus accepts only one sync-wait on CTRL-lowered
    instructions; hoist extra waits onto preceding single-wait NOPs on the
    same engine (engines execute their program in order, so this is
    semantics-preserving)."""
    for f in nc.m.functions:
        for bb in f.blocks:
            newlist = []
            changed = False
            for ins in bb.instructions:
                si = ins.sync_info
                if si is not None and len(si.on_wait) > max_keep:
                    waits = list(si.on_wait)
                    extra, kept = waits[:-max_keep], waits[-max_keep:]
                    for j, w in enumerate(extra):
                        newlist.append(
                            mybir.InstNoOp(
                                name=f"{ins.name}-presync-{j}",
                                engine=ins.engine,
                                sync_info=mybir.SyncInfo(on_wait=[w], on_update=[]),
                                bass_nofuse=True,
                            )
                        )
                    ins.sync_info = mybir.SyncInfo(
                        on_wait=kept, on_update=list(si.on_update)
                    )
                    changed = True
                newlist.append(ins)
            if changed:
                bb.instructions = newlist


def _build():
    nc = bass.Bass()
    xt = nc.dram_tensor("xt", [128, KT, T], F32, kind="ExternalInput")
    xtb = nc.dram_tensor("xtb", [128, KT, T], BF16, kind="ExternalInput")
    w1 = nc.dram_tensor("w1", [128, KT, F], BF16, kind="ExternalInput")
    w2 = nc.dram_tensor("w2", [128, FT, D], BF16, kind="ExternalInput")
    wr = nc.dram_tensor("wr", [128, KT, E], F32, kind="ExternalInput")
    b1 = nc.dram_tensor("b1", [128, FT], F32, kind="ExternalInput")
    b2 = nc.dram_tensor("b2", [1, D], BF16, kind="ExternalInput")
    esel = nc.dram_tensor("esel", [128, E], F32, kind="ExternalInput")
    out_shard = nc.dram_tensor("out_shard", [TSHARD, D], F32, kind="ExternalOutput")
    out_aux = nc.dram_tensor("out_aux", [1, 1], F32, kind="ExternalOutput")

    partial = nc.dram_tensor("partial", [T, D], F32)
    rs_out = nc.dram_tensor("rs_out", [TSHARD, D], F32)

    with tile.TileContext(nc) as tc:
        with (
            tc.tile_pool(name="wpool", bufs=1) as wpool,
            tc.tile_pool(name="big", bufs=1) as bigp,
            tc.tile_pool(name="xb", bufs=1) as xbp,
            tc.tile_pool(name="xr", bufs=2) as xrp,
            tc.tile_pool(name="ysb", bufs=2) as ysbp,
            tc.tile_pool(name="small", bufs=2) as smallp,
            tc.tile_pool(name="stat", bufs=1) as statp,
            tc.tile_pool(name="rps", bufs=2, space="PSUM") as rpsp,
            tc.tile_pool(name="hps", bufs=2, space="PSUM") as hpsp,
            tc.tile_pool(name="yps", bufs=2, space="PSUM") as ypsp,
        ):
            w1_t = wpool.tile([128, KT, F], BF16)
            w2_t = wpool.tile([128, FT, D], BF16)
            wr_t = wpool.tile([128, KT, E], F32)
            b1_t = wpool.tile([128, FT], F32)
            b2_t = wpool.tile([1, D], BF16)
            ones_t = wpool.tile([1, 128], BF16)
            ones128 = wpool.tile([128, 1], F32)
            esel_t = wpool.tile([128, E], F32)
            cw_t = wpool.tile([128, T // 128], F32)
            nc.sync.dma_start(w1_t[:], w1[:])
            nc.sync.dma_start(w2_t[:], w2[:])
            nc.sync.dma_start(wr_t[:], wr[:])
            nc.sync.dma_start(b1_t[:], b1[:])
            nc.sync.dma_start(b2_t[:], b2[:])
            nc.sync.dma_start(esel_t[:], esel[:])
            nc.vector.memset(ones_t[:], 1.0)
            nc.vector.memset(ones128[:], 1.0)

            probs_acc = statp.tile([128, E], F32)
            cnt_acc = statp.tile([128, E], F32)
            nc.vector.memset(probs_acc[:], 0.0)
            nc.vector.memset(cnt_acc[:], 0.0)

            # ---- FFN (bf16) for this core's expert over all tokens
            for ch in range(NCH):
                xb = xbp.tile([128, KT, TCH], BF16)
                nc.sync.dma_start(xb[:], xtb[:, :, ch * TCH:(ch + 1) * TCH])
                # router for this chunk's 4 token tiles (fp32), overlapped
                # with the PE-heavy FFN below
                for tsub in range(TCH // 128):
                    tt = ch * (TCH // 128) + tsub
                    xr = xrp.tile([128, KT, 128], F32, tag="xr")
                    nc.sync.dma_start(
                        xr[:], xt[:, :, tt * 128:(tt + 1) * 128]
                    )
                    ps = rpsp.tile([128, E], F32)
                    for kt in range(KT):
                        nc.tensor.matmul(
                            ps[:],
                            xr[:, kt, :],
                            wr_t[:, kt, :],
                            start=(kt == 0),
                            stop=(kt == KT - 1),
                        )
                    probs = smallp.tile([128, E], F32, tag="probs")
                    m1 = smallp.tile([128, 1], F32, tag="m1")
                    tmp8 = smallp.tile([128, E], F32, tag="tmp8")
                    tmp1 = smallp.tile([128, 1], F32, tag="tmp1")
                    nc.vector.reduce_max(m1[:], ps[:], axis=mybir.AxisListType.X)
                    nc.vector.tensor_scalar_mul(tmp1[:], m1[:], -1.0)
                    nc.scalar.activation(probs[:], ps[:], AF.Exp, bias=tmp1[:])
                    nc.vector.reduce_sum(tmp1[:], probs[:], axis=mybir.AxisListType.X)
                    nc.vector.reciprocal(tmp1[:], tmp1[:])
                    nc.vector.tensor_scalar_mul(probs[:], probs[:], tmp1[:])
                    nc.vector.reduce_max(m1[:], probs[:], axis=mybir.AxisListType.X)
                    nc.vector.tensor_scalar(
                        tmp8[:], probs[:], m1[:], 1e30, op0=ALU.is_ge, op1=ALU.mult
                    )
                    nc.vector.tensor_tensor(tmp8[:], probs[:], tmp8[:], ALU.subtract)
                    m2 = smallp.tile([128, 1], F32, tag="m2")
                    nc.vector.reduce_max(m2[:], tmp8[:], axis=mybir.AxisListType.X)
                    ge2 = smallp.tile([128, E], F32, tag="ge2")
                    nc.vector.tensor_scalar(
                        ge2[:], probs[:], m2[:], None, op0=ALU.is_ge
                    )
                    nc.vector.tensor_tensor(m1[:], m1[:], m2[:], ALU.add)
                    nc.vector.reciprocal(m1[:], m1[:])
                    cwall = smallp.tile([128, E], F32, tag="cwall")
                    nc.vector.tensor_tensor(cwall[:], probs[:], ge2[:], ALU.mult)
                    nc.vector.tensor_scalar_mul(cwall[:], cwall[:], m1[:])
                    nc.vector.tensor_tensor(
                        probs_acc[:], probs_acc[:], probs[:], ALU.add
                    )
                    nc.vector.tensor_tensor(cnt_acc[:], cnt_acc[:], ge2[:], ALU.add)
                    nc.vector.tensor_tensor(cwall[:], cwall[:], esel_t[:], ALU.mult)
                    nc.vector.reduce_sum(
                        cw_t[:, tt:tt + 1], cwall[:], axis=mybir.AxisListType.X
                    )
                hT = bigp.tile([128, FT, TCH], BF16, tag="bigslot")
                for ft in range(FT):
                    hp = hpsp.tile([128, TCH], F32)
                    for kt in range(KT):
                        nc.tensor.matmul(
                            hp[:],
                            w1_t[:, kt, ft * 128:(ft + 1) * 128],
                            xb[:, kt, :],
                            start=(kt == 0),
                            stop=(kt == KT - 1),
                        )
                    nc.scalar.activation(
                        hT[:, ft, :], hp[:], AF.Gelu, bias=b1_t[:, ft:ft + 1]
                    )
                for tsub in range(TCH // 128):
                    tt = ch * (TCH // 128) + tsub
                    yp = ypsp.tile([128, D], F32)
                    nhalf = D // 512 if D >= 512 else 1
                    nsz = D // nhalf
                    for ftk in range(FT):
                        for h in range(nhalf):
                            nc.tensor.matmul(
                                yp[:, h * nsz:(h + 1) * nsz],
                                hT[:, ftk, tsub * 128:(tsub + 1) * 128],
                                w2_t[:, ftk, h * nsz:(h + 1) * nsz],
                                start=(ftk == 0),
                                stop=False,
                            )
                    for h in range(nhalf):
                        nc.tensor.matmul(
                            yp[:, h * nsz:(h + 1) * nsz],
                            ones_t[:],
                            b2_t[:, h * nsz:(h + 1) * nsz],
                            start=False,
                            stop=True,
                        )
                    ysb = ysbp.tile([128, D], F32)
                    nc.vector.tensor_scalar_mul(ysb[:], yp[:], cw_t[:, tt:tt + 1])
                    nc.sync.dma_start(partial[tt * 128:(tt + 1) * 128, :], ysb[:])

            # ---- aux loss: 0.01 * E * sum_e mean_probs[e] * counts[e]/(2T)
            col = rpsp.tile([1, E], F32, tag="ps")
            col2 = rpsp.tile([1, E], F32, tag="ps")
            nc.tensor.matmul(col[:], ones128[:], probs_acc[:], start=True, stop=True)
            nc.tensor.matmul(col2[:], ones128[:], cnt_acc[:], start=True, stop=True)
            auxv = smallp.tile([1, E], F32, tag="auxv")
            auxv2 = smallp.tile([1, E], F32, tag="auxv2")
            aux1 = smallp.tile([1, 1], F32, tag="aux1")
            nc.vector.tensor_copy(auxv2[:], col2[:])
            nc.vector.tensor_tensor(auxv[:], col[:], auxv2[:], ALU.mult)
            nc.vector.reduce_sum(aux1[:], auxv[:], axis=mybir.AxisListType.X)
            nc.vector.tensor_scalar_mul(aux1[:], aux1[:], 0.01 * E / (T * (T * 2.0)))
            nc.sync.dma_start(out_aux[:], aux1[:])

            # ---- combine partials across cores
            nc.gpsimd.collective_compute(
                "ReduceScatter",
                ALU.add,
                replica_groups=[list(range(N_CORES))],
                ins=[partial.ap().opt()],
                outs=[rs_out.ap().opt()],
            )
            nc.sync.dma_start(out_shard[:], rs_out[:])

    return nc


class _Runner:
    """Compile once, keep the jitted SPMD callable + device inputs."""

    def __init__(self, nc, n_cores):
        import jax
        from jax.sharding import Mesh, PartitionSpec
        from jax.experimental.shard_map import shard_map
        from concourse import bass2jax
        from concourse.bass2jax import _bass_exec_p, partition_id_tensor

        bass2jax.install_neuronx_cc_hook()
        self.jax = jax
        self.n_cores = n_cores
        self.PartitionSpec = PartitionSpec

        partition_name = (
            nc.partition_id_tensor.name if nc.partition_id_tensor else None
        )
        in_names, out_names, out_avals, zero_outs = [], [], [], []
        for alloc in nc.m.functions[0].allocations:
            if not isinstance(alloc, mybir.MemoryLocationSet):
                continue
            name = alloc.memorylocations[0].name
            if alloc.kind == "ExternalInput":
                if name != partition_name:
                    in_names.append(name)
            elif alloc.kind == "ExternalOutput":
                shape = tuple(alloc.tensor_shape)
                dtype = mybir.dt.np(alloc.dtype)
                out_names.append(name)
                out_avals.append(jax.core.ShapedArray(shape, dtype))
                zero_outs.append(np.zeros(shape, dtype))
        self.in_names = in_names
        self.out_names = out_names
        self.out_avals = out_avals
        self.zero_outs = zero_outs
        n_params = len(in_names)
        all_in_names = list(in_names) + list(out_names)
        if partition_name is not None:
            all_in_names.append(partition_name)

        def _body(*args):
            operands = list(args)
            if partition_name is not None:
                operands.append(partition_id_tensor())
            outs = _bass_exec_p.bind(
                *operands,
                out_avals=tuple(out_avals),
                in_names=tuple(all_in_names),
                out_names=tuple(out_names),
                lowering_input_output_aliases=(),
                sim_require_finite=True,
                sim_require_nnan=True,
                nc=nc,
            )
            return tuple(outs)

        devices = jax.devices()[:n_cores]
        assert len(devices) == n_cores, (
            f"need {n_cores} neuron cores, found {len(jax.devices())}"
        )
        self.mesh = Mesh(np.asarray(devices), ("core",))
        in_specs = (PartitionSpec("core"),) * (n_params + len(out_names))
        out_specs = (PartitionSpec("core"),) * len(out_names)
        self.fn = jax.jit(
            shard_map(
                _body,
                mesh=self.mesh,
                in_specs=in_specs,
                out_specs=out_specs,
                check_rep=False,
            ),
            keep_unused=True,
        )

    def run(self, in_maps):
        jax = self.jax
        n = self.n_cores
        arrs = []
        for name in self.in_names:
            arrs.append(
                np.concatenate([np.asarray(in_maps[c][name]) for c in range(n)], 0)
            )
        for z in self.zero_outs:
            arrs.append(np.zeros((n * z.shape[0], *z.shape[1:]), z.dtype))
        sharding = jax.sharding.NamedSharding(self.mesh, self.PartitionSpec("core"))
        dev_args = [jax.device_put(a, sharding) for a in arrs]
        outs = self.fn(*dev_args)
        jax.block_until_ready(outs)
        return [
            {
                name: np.asarray(outs[i]).reshape(n, *self.out_avals[i].shape)[c]
                for i, name in enumerate(self.out_names)
            }
            for c in range(n)
        ]


_RUNNER = None


def _get_runner():
    global _RUNNER
    if _RUNNER is None:
        nc = _build()
        _split_excess_waits(nc)
        _RUNNER = _Runner(nc, N_CORES)
    return _RUNNER


def kernel(x, W_router, W1, b1, W2, b2):
    x = np.asarray(x, dtype=np.float32)
    W_router = np.asarray(W_router, dtype=np.float32)
    W1 = np.asarray(W1, dtype=np.float32)
    b1 = np.asarray(b1, dtype=np.float32)
    W2 = np.asarray(W2, dtype=np.float32)
    b2 = np.asarray(b2, dtype=np.float32)
    B, S, _ = x.shape

    xf = np.ascontiguousarray(x.reshape(T, D))
    xt = np.ascontiguousarray(xf.reshape(T, KT, 128).transpose(2, 1, 0))
    xtb = xt.astype(ml_dtypes.bfloat16)
    wr = np.ascontiguousarray(
        W_router.reshape(KT, 128, E).transpose(1, 0, 2)
    ).astype(np.float32)
    in_maps = []
    for c in range(N_CORES):
        w1c = np.ascontiguousarray(
            W1[c].reshape(KT, 128, F).transpose(1, 0, 2)
        ).astype(ml_dtypes.bfloat16)
        w2c = np.ascontiguousarray(
            W2[c].reshape(FT, 128, D).transpose(1, 0, 2)
        ).astype(ml_dtypes.bfloat16)
        b1c = np.ascontiguousarray(b1[c].reshape(FT, 128).T).astype(np.float32)
        b2c = b2[c].reshape(1, D).astype(ml_dtypes.bfloat16)
        es = np.zeros((128, E), np.float32)
        es[:, c % E] = 1.0
        in_maps.append(
            {
                "xt": xt,
                "xtb": xtb,
                "w1": w1c,
                "w2": w2c,
                "wr": wr,
                "b1": b1c,
                "b2": b2c,
                "esel": es,
            }
        )

    results = _get_runner().run(in_maps)
    out = np.concatenate([r["out_shard"] for r in results], axis=0)
    aux = np.float32(results[0]["out_aux"][0, 0])
    return out.reshape(B, S, D), aux


# revision 5
# speedup vs baseline: 3.4780x; 3.4780x over previous
"""MoE layer (top-2 of 8 experts, T=4096, D=1024, F=4096) on 8 Trainium2
NeuronCores, expert-parallel.

Sharding: x is replicated to all cores; core c holds expert c's weights.
Each core computes the router in fp32 (token routing must match the fp32
reference bit-for-bit at the top-k decision), runs its expert's FFN in bf16
over all tokens, scales each token row by its combine weight (zero when the
token is not routed to this expert), and an fp32 ReduceScatter sums the
per-expert partials so core c ends up with token rows [512c, 512c+512).
The host concatenates the 8 shards. The load-balance aux loss is computed
on-device (identically on every core) from per-expert prob sums and top-2
counts.

Self-contained: only needs numpy / ml_dtypes / concourse (the Bass stack
present in the execution container).
"""
import numpy as np
import ml_dtypes

import concourse.bass as bass
import concourse.mybir as mybir
import concourse.tile as tile

F32 = mybir.dt.float32
BF16 = mybir.dt.bfloat16
AF = mybir.ActivationFunctionType
ALU = mybir.AluOpType

# problem dims (hardcoded per task spec)
T, D, F, E = 4096, 1024, 4096, 8
N_CORES = 8
KT = D // 128          # contraction tiles for x @ W1 and the router
FT = F // 128          # contraction tiles for h @ W2
TCH = 512              # tokens per FFN chunk
NCH = T // TCH
TSHARD = T // N_CORES


def _split_excess_waits(nc, max_keep=1):
    """This container's walrus accepts only one sync-wait on CTRL-lowered
    instructions; hoist extra waits onto preceding single-wait NOPs on the
    same engine (engines execute their program in order, so this is
    semantics-preserving)."""
    for f in nc.m.functions:
        for bb in f.blocks:
            newlist = []
            changed = False
            for ins in bb.instructions:
                si = ins.sync_info
                if si is not None and len(si.on_wait) > max_keep:
                    waits = list(si.on_wait)
                    extra, kept = waits[:-max_keep], waits[-max_keep:]
                    for j, w in enumerate(extra):
                        newlist.append(
                            mybir.InstNoOp(
                                name=f"{ins.name}-presync-{j}",
                                engine=ins.engine,
                                sync_info=mybir.SyncInfo(on_wait=[w], on_update=[]),
                                bass_nofuse=True,
                            )
                        )
                    ins.sync_info = mybir.SyncInfo(
                        on_wait=kept, on_update=list(si.on_update)
                    )
                    changed = True
                newlist.append(ins)
            if changed:
                bb.instructions = newlist


def _build():
    nc = bass.Bass()
    xt = nc.dram_tensor("xt", [128, KT, T], F32, kind="ExternalInput")
    xtb = nc.dram_tensor("xtb", [128, KT, T], BF16, kind="ExternalInput")
    w1 = nc.dram_tensor("w1", [128, KT, F], BF16, kind="ExternalInput")
    w2 = nc.dram_tensor("w2", [128, FT, D], BF16, kind="ExternalInput")
    wr = nc.dram_tensor("wr", [128, KT, E], F32, kind="ExternalInput")
    b1 = nc.dram_tensor("b1", [128, FT], F32, kind="ExternalInput")
    b2 = nc.dram_tensor("b2", [1, D], BF16, kind="ExternalInput")
    esel = nc.dram_tensor("esel", [128, E], F32, kind="ExternalInput")
    out_shard = nc.dram_tensor("out_shard", [4, TSHARD // 4, D], F32, kind="ExternalOutput")
    out_aux = nc.dram_tensor("out_aux", [1, 1], F32, kind="ExternalOutput")

    partial = nc.dram_tensor("partial", [T, D], F32)
    rs_out = nc.dram_tensor("rs_out", [4, TSHARD // 4, D], F32)

    with tile.TileContext(nc) as tc:
        with (
            tc.tile_pool(name="wpool", bufs=1) as wpool,
            tc.tile_pool(name="big", bufs=1) as bigp,
            tc.tile_pool(name="xb", bufs=1) as xbp,
            tc.tile_pool(name="xr", bufs=2) as xrp,
            tc.tile_pool(name="ysb", bufs=2) as ysbp,
            tc.tile_pool(name="small", bufs=2) as smallp,
            tc.tile_pool(name="stat", bufs=1) as statp,
            tc.tile_pool(name="rps", bufs=2, space="PSUM") as rpsp,
            tc.tile_pool(name="hps", bufs=2, space="PSUM") as hpsp,
            tc.tile_pool(name="yps", bufs=2, space="PSUM") as ypsp,
        ):
            w1_t = wpool.tile([128, KT, F], BF16)
            w2_t = wpool.tile([128, FT, D], BF16)
            wr_t = wpool.tile([128, KT, E], F32)
            b1_t = wpool.tile([128, FT], F32)
            b2_t = wpool.tile([1, D], BF16)
            ones_t = wpool.tile([1, 128], BF16)
            ones128 = wpool.tile([128, 1], F32)
            esel_t = wpool.tile([128, E], F32)
            cw_t = wpool.tile([128, T // 128], F32)
            nc.sync.dma_start(w1_t[:], w1[:])
            nc.sync.dma_start(w2_t[:], w2[:])
            nc.sync.dma_start(wr_t[:], wr[:])
            nc.sync.dma_start(b1_t[:], b1[:])
            nc.sync.dma_start(b2_t[:], b2[:])
            nc.sync.dma_start(esel_t[:], esel[:])
            nc.vector.memset(ones_t[:], 1.0)
            nc.vector.memset(ones128[:], 1.0)

            probs_acc = statp.tile([128, E], F32)
            cnt_acc = statp.tile([128, E], F32)
            nc.vector.memset(probs_acc[:], 0.0)
            nc.vector.memset(cnt_acc[:], 0.0)

            # ---- FFN (bf16) for this core's expert over all tokens
            for ch in range(NCH):
                xb = xbp.tile([128, KT, TCH], BF16)
                nc.sync.dma_start(xb[:], xtb[:, :, ch * TCH:(ch + 1) * TCH])
                # router for this chunk's 4 token tiles (fp32), overlapped
                # with the PE-heavy FFN below
                for tsub in range(TCH // 128):
                    tt = ch * (TCH // 128) + tsub
                    xr = xrp.tile([128, KT, 128], F32, tag="xr")
                    nc.sync.dma_start(
                        xr[:], xt[:, :, tt * 128:(tt + 1) * 128]
                    )
                    ps = rpsp.tile([128, E], F32)
                    for kt in range(KT):
                        nc.tensor.matmul(
                            ps[:],
                            xr[:, kt, :],
                            wr_t[:, kt, :],
                            start=(kt == 0),
                            stop=(kt == KT - 1),
                        )
                    probs = smallp.tile([128, E], F32, tag="probs")
                    m1 = smallp.tile([128, 1], F32, tag="m1")
                    tmp8 = smallp.tile([128, E], F32, tag="tmp8")
                    tmp1 = smallp.tile([128, 1], F32, tag="tmp1")
                    nc.vector.reduce_max(m1[:], ps[:], axis=mybir.AxisListType.X)
                    nc.vector.tensor_scalar_mul(tmp1[:], m1[:], -1.0)
                    nc.scalar.activation(probs[:], ps[:], AF.Exp, bias=tmp1[:])
                    nc.vector.reduce_sum(tmp1[:], probs[:], axis=mybir.AxisListType.X)
                    nc.vector.reciprocal(tmp1[:], tmp1[:])
                    nc.vector.tensor_scalar_mul(probs[:], probs[:], tmp1[:])
                    nc.vector.reduce_max(m1[:], probs[:], axis=mybir.AxisListType.X)
                    nc.vector.tensor_scalar(
                        tmp8[:], probs[:], m1[:], 1e30, op0=ALU.is_ge, op1=ALU.mult
                    )
                    nc.vector.tensor_tensor(tmp8[:], probs[:], tmp8[:], ALU.subtract)
                    m2 = smallp.tile([128, 1], F32, tag="m2")
                    nc.vector.reduce_max(m2[:], tmp8[:], axis=mybir.AxisListType.X)
                    ge2 = smallp.tile([128, E], F32, tag="ge2")
                    nc.vector.tensor_scalar(
                        ge2[:], probs[:], m2[:], None, op0=ALU.is_ge
                    )
                    nc.vector.tensor_tensor(m1[:], m1[:], m2[:], ALU.add)
                    nc.vector.reciprocal(m1[:], m1[:])
                    cwall = smallp.tile([128, E], F32, tag="cwall")
                    nc.vector.tensor_tensor(cwall[:], probs[:], ge2[:], ALU.mult)
                    nc.vector.tensor_scalar_mul(cwall[:], cwall[:], m1[:])
                    nc.vector.tensor_tensor(
                        probs_acc[:], probs_acc[:], probs[:], ALU.add
                    )
                    nc.vector.tensor_tensor(cnt_acc[:], cnt_acc[:], ge2[:], ALU.add)
                    nc.vector.tensor_tensor(cwall[:], cwall[:], esel_t[:], ALU.mult)
                    nc.vector.reduce_sum(
                        cw_t[:, tt:tt + 1], cwall[:], axis=mybir.AxisListType.X
                    )
                hT = bigp.tile([128, FT, TCH], BF16, tag="bigslot")
                for ft in range(FT):
                    hp = hpsp.tile([128, TCH], F32)
                    for kt in range(KT):
                        nc.tensor.matmul(
                            hp[:],
                            w1_t[:, kt, ft * 128:(ft + 1) * 128],
                            xb[:, kt, :],
                            start=(kt == 0),
                            stop=(kt == KT - 1),
                        )
                    nc.scalar.activation(
                        hT[:, ft, :], hp[:], AF.Gelu, bias=b1_t[:, ft:ft + 1]
                    )
                for tsub in range(TCH // 128):
                    tt = ch * (TCH // 128) + tsub
                    yp = ypsp.tile([128, D], F32)
                    nhalf = D // 512 if D >= 512 else 1
                    nsz = D // nhalf
                    for ftk in range(FT):
                        for h in range(nhalf):
                            nc.tensor.matmul(
                                yp[:, h * nsz:(h + 1) * nsz],
                                hT[:, ftk, tsub * 128:(tsub + 1) * 128],
                                w2_t[:, ftk, h * nsz:(h + 1) * nsz],
                                start=(ftk == 0),
                                stop=False,
                            )
                    for h in range(nhalf):
                        nc.tensor.matmul(
                            yp[:, h * nsz:(h + 1) * nsz],
                            ones_t[:],
                            b2_t[:, h * nsz:(h + 1) * nsz],
                            start=False,
                            stop=True,
                        )
                    ysb = ysbp.tile([128, D], F32)
                    nc.vector.tensor_scalar_mul(ysb[:], yp[:], cw_t[:, tt:tt + 1])
                    nc.sync.dma_start(partial[tt * 128:(tt + 1) * 128, :], ysb[:])

            # ---- aux loss: 0.01 * E * sum_e mean_probs[e] * counts[e]/(2T)
            col = rpsp.tile([1, E], F32, tag="ps")
            col2 = rpsp.tile([1, E], F32, tag="ps")
            nc.tensor.matmul(col[:], ones128[:], probs_acc[:], start=True, stop=True)
            nc.tensor.matmul(col2[:], ones128[:], cnt_acc[:], start=True, stop=True)
            auxv = smallp.tile([1, E], F32, tag="auxv")
            auxv2 = smallp.tile([1, E], F32, tag="auxv2")
            aux1 = smallp.tile([1, 1], F32, tag="aux1")
            nc.vector.tensor_copy(auxv2[:], col2[:])
            nc.vector.tensor_tensor(auxv[:], col[:], auxv2[:], ALU.mult)
            nc.vector.reduce_sum(aux1[:], auxv[:], axis=mybir.AxisListType.X)
            nc.vector.tensor_scalar_mul(aux1[:], aux1[:], 0.01 * E / (T * (T * 2.0)))
            nc.sync.dma_start(out_aux[:], aux1[:])

            # ---- combine partials across cores: 4 chunked ReduceScatters,
            # each eligible as soon as its quarter of `partial` is written,
            # so the first three overlap with the remaining FFN compute.
            for k in range(4):
                nc.gpsimd.collective_compute(
                    "ReduceScatter",
                    ALU.add,
                    replica_groups=[list(range(N_CORES))],
                    ins=[partial[k * (T // 4):(k + 1) * (T // 4), :].opt()],
                    outs=[rs_out[k].opt()],
                )
                nc.sync.dma_start(out_shard[k], rs_out[k])

    return nc


class _Runner:
    """Compile once, keep the jitted SPMD callable + device inputs."""

    def __init__(self, nc, n_cores):
        import jax
        from jax.sharding import Mesh, PartitionSpec
        from jax.experimental.shard_map import shard_map
        from concourse import bass2jax
        from concourse.bass2jax import _bass_exec_p, partition_id_tensor

        bass2jax.install_neuronx_cc_hook()
        self.jax = jax
        self.n_cores = n_cores
        self.PartitionSpec = PartitionSpec

        partition_name = (
            nc.partition_id_tensor.name if nc.partition_id_tensor else None
        )
        in_names, out_names, out_avals, zero_outs = [], [], [], []
        for alloc in nc.m.functions[0].allocations:
            if not isinstance(alloc, mybir.MemoryLocationSet):
                continue
            name = alloc.memorylocations[0].name
            if alloc.kind == "ExternalInput":
                if name != partition_name:
                    in_names.append(name)
            elif alloc.kind == "ExternalOutput":
                shape = tuple(alloc.tensor_shape)
                dtype = mybir.dt.np(alloc.dtype)
                out_names.append(name)
                out_avals.append(jax.core.ShapedArray(shape, dtype))
                zero_outs.append(np.zeros(shape, dtype))
        self.in_names = in_names
        self.out_names = out_names
        self.out_avals = out_avals
        self.zero_outs = zero_outs
        n_params = len(in_names)
        all_in_names = list(in_names) + list(out_names)
        if partition_name is not None:
            all_in_names.append(partition_name)

        def _body(*args):
            operands = list(args)
            if partition_name is not None:
                operands.append(partition_id_tensor())
            outs = _bass_exec_p.bind(
                *operands,
                out_avals=tuple(out_avals),
                in_names=tuple(all_in_names),
                out_names=tuple(out_names),
                lowering_input_output_aliases=(),
                sim_require_finite=True,
                sim_require_nnan=True,
                nc=nc,
            )
            return tuple(outs)

        devices = jax.devices()[:n_cores]
        assert len(devices) == n_cores, (
            f"need {n_cores} neuron cores, found {len(jax.devices())}"
        )
        self.mesh = Mesh(np.asarray(devices), ("core",))
        in_specs = (PartitionSpec("core"),) * (n_params + len(out_names))
        out_specs = (PartitionSpec("core"),) * len(out_names)
        self.fn = jax.jit(
            shard_map(
                _body,
                mesh=self.mesh,
                in_specs=in_specs,
                out_specs=out_specs,
                check_rep=False,
            ),
            keep_unused=True,
        )

    def run(self, in_maps):
        jax = self.jax
        n = self.n_cores
        arrs = []
        for name in self.in_names:
            arrs.append(
                np.concatenate([np.asarray(in_maps[c][name]) for c in range(n)], 0)
            )
        for z in self.zero_outs:
            arrs.append(np.zeros((n * z.shape[0], *z.shape[1:]), z.dtype))
        sharding = jax.sharding.NamedSharding(self.mesh, self.PartitionSpec("core"))
        dev_args = [jax.device_put(a, sharding) for a in arrs]
        outs = self.fn(*dev_args)
        jax.block_until_ready(outs)
        return [
            {
                name: np.asarray(outs[i]).reshape(n, *self.out_avals[i].shape)[c]
                for i, name in enumerate(self.out_names)
            }
            for c in range(n)
        ]


_RUNNER = None


def _get_runner():
    global _RUNNER
    if _RUNNER is None:
        nc = _build()
        _split_excess_waits(nc)
        _RUNNER = _Runner(nc, N_CORES)
    return _RUNNER


def kernel(x, W_router, W1, b1, W2, b2):
    x = np.asarray(x, dtype=np.float32)
    W_router = np.asarray(W_router, dtype=np.float32)
    W1 = np.asarray(W1, dtype=np.float32)
    b1 = np.asarray(b1, dtype=np.float32)
    W2 = np.asarray(W2, dtype=np.float32)
    b2 = np.asarray(b2, dtype=np.float32)
    B, S, _ = x.shape

    xf = np.ascontiguousarray(x.reshape(T, D))
    xt = np.ascontiguousarray(xf.reshape(T, KT, 128).transpose(2, 1, 0))
    xtb = xt.astype(ml_dtypes.bfloat16)
    wr = np.ascontiguousarray(
        W_router.reshape(KT, 128, E).transpose(1, 0, 2)
    ).astype(np.float32)
    in_maps = []
    for c in range(N_CORES):
        w1c = np.ascontiguousarray(
            W1[c].reshape(KT, 128, F).transpose(1, 0, 2)
        ).astype(ml_dtypes.bfloat16)
        w2c = np.ascontiguousarray(
            W2[c].reshape(FT, 128, D).transpose(1, 0, 2)
        ).astype(ml_dtypes.bfloat16)
        b1c = np.ascontiguousarray(b1[c].reshape(FT, 128).T).astype(np.float32)
        b2c = b2[c].reshape(1, D).astype(ml_dtypes.bfloat16)
        es = np.zeros((128, E), np.float32)
        es[:, c % E] = 1.0
        in_maps.append(
            {
                "xt": xt,
                "xtb": xtb,
                "w1": w1c,
                "w2": w2c,
                "wr": wr,
                "b1": b1c,
                "b2": b2c,
                "esel": es,
            }
        )

    results = _get_runner().run(in_maps)
    out = np.empty((T, D), np.float32)
    cl = TSHARD // 4
    for c in range(N_CORES):
        sh = results[c]["out_shard"]
        for k in range(4):
            base = k * (T // 4) + c * cl
            out[base:base + cl] = sh[k]
    aux = np.float32(results[0]["out_aux"][0, 0])
    return out.reshape(B, S, D), aux
